# revision 16
# baseline (speedup 1.0000x reference)
"""CPDecoding (embedding_lookup) Trainium2 kernel.

out[n] = sum_c fz[c,n]*fy[c,n]*fx[c,n], where f* is a 1-D linear
interpolation (grid_sample, align_corners=True) of a (96, 512) line table
at per-point coordinates.

Per core (98304 points, z-sorted on host, flat order n = 2048*chunk + j):
  - y/x axes: 64x pre-upsampled fp16 tables (linear interp pre-evaluated,
    so no interp FMA on device); dma_gather(transpose=True) lands rows in
    c-layout [component-partition, point-free]. m = gy*gx on DVE (one
    fp16 2x multiply -- no other elementwise work).
  - z axis + the 96-component reduction fused on the PE: points are
    sorted by iz = floor(pos_z), so each 128-point tile spans <= 2
    consecutive z rows. Per tile: uv[r, :] = sum_c Lz[c, a_t+r] * m[c, :]
    (K=96 matmul, M=4), with the 4-row slab fetched once per tile by a
    small slab dma_gather. A tiny PE transpose puts uv in pt-layout and
    DVE dots it with 4-wide interp weights
    W[p,f,r] = (r==d)*(1-w) + (r==d+1)*w.  No z gather, no reduce tree.
  - The z-side setup (index chains, weight build, slab fetch) is emitted
    in slices across the first chunk bodies so the y/x gather stream
    starts within a few microseconds.
  - Host un-permutes the output.
"""

import numpy as np

N_CORES = 8
N_TOTAL = 4096 * 192
N_CORE = N_TOTAL // N_CORES      # 98304 points per core
P = 128
F = N_CORE // P                  # 768 tiles of 128 points
GROUPS = 8
PHI = N_CORE // 16 // GROUPS     # 768 idx-cols per group (wrapped-16)
C = 96
R = 512
S = 64                           # y/x upsample factor
NQ = (R - 1) * S + 1             # 32705
ELEM = 256                       # fp16 elems per row (512B, transpose mode)
K = 4                            # z slab rows per tile
RPAD = R + 8                     # z table rows incl. OOB pad
CHUNK_T = 16                     # tiles per chunk
CHUNK_PTS = P * CHUNK_T          # 2048 points
N_CHUNKS = F // CHUNK_T          # 48
CHUNKS_PER_GROUP = N_CHUNKS // GROUPS  # 6
SUBCOLS = CHUNK_PTS // 16        # 128 idx cols per chunk
QSCALE = (R - 1) * S
NSLAB = K * F                    # 3072 slab-row fetches
SLABCOLS = NSLAB // 16           # 192

_BUILT = None


def _build_nc():
    import concourse.bacc as bacc
    import concourse.tile as tile
    from concourse import mybir
    from concourse.library_config import mlp as lib_mlp
    from concourse.masks import make_identity

    dt = mybir.dt
    Alu = mybir.AluOpType

    nc = bacc.Bacc("TRN2", target_bir_lowering=False, debug=False,
                   num_devices=N_CORES, num_swdge_queues=1)

    pw16 = nc.dram_tensor("pw16", [P, PHI * 2], dt.float32,
                          kind="ExternalInput").ap()         # y,x wrapped-16
    pwz = nc.dram_tensor("pwz", [P, F], dt.float32,
                         kind="ExternalInput").ap()          # z pt-layout
    tby = nc.dram_tensor("tby", [NQ, ELEM], dt.float16,
                         kind="ExternalInput").ap()
    tbx = nc.dram_tensor("tbx", [NQ, ELEM], dt.float16,
                         kind="ExternalInput").ap()
    tbz = nc.dram_tensor("tbz", [RPAD, ELEM], dt.float16,
                         kind="ExternalInput").ap()          # exact z rows
    consts = nc.dram_tensor("consts", [P, 10], dt.float16,
                            kind="ExternalInput").ap()
    rcon = nc.dram_tensor("rcon", [16, 1], dt.int16,
                          kind="ExternalInput").ap()
    onesrow = nc.dram_tensor("onesrow", [1, P], dt.float16,
                             kind="ExternalInput").ap()
    out_d = nc.dram_tensor("out", [P, F], dt.float32,
                           kind="ExternalOutput").ap()

    with tile.TileContext(nc) as tc:
        with tc.tile_pool(name="persist", bufs=1) as pp:
            idx_all = pp.tile([P, 2, PHI], dt.int16, tag="idx")
            out_full = pp.tile([P, F], dt.float32, tag="out")
            Wpt = pp.tile([P, F, K], dt.float16, tag="wpt")
            slabT = pp.tile([P, 2, NSLAB], dt.float16, tag="slabT")
            slabidx = pp.tile([P, SLABCOLS], dt.int16, tag="slabidx")
            cst = pp.tile([P, 10], dt.float16, tag="cst")
            nc.sync.dma_start(cst[:], consts)
            rc = pp.tile([16, 1], dt.int16, tag="rcon")
            nc.sync.dma_start(rc[:], rcon)
            ones1 = pp.tile([1, P], dt.float16, tag="ones1")
            nc.sync.dma_start(ones1[:], onesrow)
            ident = pp.tile([P, P], dt.float16, tag="ident")
            make_identity(nc, ident[:])

            def ptile(nm, shape, dtype=dt.float32):
                return pp.tile(shape, dtype, tag=nm, name=nm)

            def floor_chain(pool_tmp, src, shape, scale, bias, hi):
                """clamp(floor(src*scale + bias), 0, hi) as fp32."""
                pos = pool_tmp(f"pos{shape[-1]}_{hi}", shape)
                nc.vector.tensor_scalar(pos[:], src, scale, bias,
                                        Alu.mult, Alu.add)
                ii = pool_tmp(f"ii{hi}", shape, dt.int32)
                nc.vector.tensor_copy(ii[:], pos[:])
                i0f = pool_tmp(f"i0f{hi}", shape)
                nc.vector.tensor_copy(i0f[:], ii[:])
                neg = pool_tmp(f"neg{hi}", shape)
                nc.vector.tensor_tensor(neg[:], pos[:], i0f[:], Alu.is_lt)
                i0a = pool_tmp(f"i0a{hi}", shape)
                nc.vector.tensor_sub(i0a[:], i0f[:], neg[:])
                i0c = pool_tmp(f"i0c{hi}", shape)
                nc.vector.tensor_scalar(i0c[:], i0a[:], float(hi), 0.0,
                                        Alu.min, Alu.max)
                return pos, i0c

            # ---------- upfront: y/x upsampled indices (wrapped-16) -------
            # First 128 idx-cols (enough for chunk (grp0, sub0)) are chained
            # and staged separately so the first gather launches early.
            SL = SUBCOLS
            with tc.tile_pool(name="setup", bufs=1) as sp:
                def stmp(nm, shape, dtype=dt.float32):
                    return sp.tile(shape, dtype, tag=nm, name=nm)

                s16 = stmp("c16", [P, PHI * 2])
                nc.sync.dma_start(s16[:], pw16)
                s3 = s16[:].rearrange("p (f k) -> p f k", k=2)
                _, q64a = floor_chain(stmp, s3[:, 0:SL, :], [P, SL * 2],
                                      QSCALE / 2.0, QSCALE / 2.0 + 0.5,
                                      NQ - 1)
                qa3 = q64a[:].rearrange("p (f k) -> p f k", k=2)
                for a in range(2):
                    nc.vector.tensor_copy(idx_all[:, a, 0:SL], qa3[:, :, a])
                _, q64b = floor_chain(stmp, s3[:, SL:PHI, :],
                                      [P, (PHI - SL) * 2],
                                      QSCALE / 2.0, QSCALE / 2.0 + 0.5,
                                      NQ - 1)
                qb3 = q64b[:].rearrange("p (f k) -> p f k", k=2)
                for a in range(2):
                    nc.vector.tensor_copy(idx_all[:, a, SL:PHI],
                                          qb3[:, :, a])

            # ---------- z-side setup, emitted in slices ----------
            zstate = {}

            def z_piece_0():
                sz = ptile("cz", [P, F])
                nc.sync.dma_start(sz[:], pwz)
                t1z = ptile("t1z", [P, F])
                nc.vector.tensor_scalar(t1z[:], sz[:], 1.0, 0.5,
                                        Alu.add, Alu.mult)
                posz, i0zf = floor_chain(ptile, t1z[:], [P, F],
                                         float(R - 1), 0.0, R - 2)
                wz = ptile("wz", [P, F])
                nc.vector.tensor_sub(wz[:], posz[:], i0zf[:])
                zstate.update(i0zf=i0zf, wz=wz)

            def z_piece_1(aps_pool):
                i0zf = zstate["i0zf"]
                af16 = ptile("af16", [1, F], dt.float16)
                nc.vector.tensor_copy(af16[:], i0zf[0:1, :])
                ai16 = ptile("ai16", [1, F], dt.int16)
                nc.vector.tensor_copy(ai16[:], i0zf[0:1, :])
                a_ps = aps_pool.tile([P, F], dt.float32, tag="aps")
                for lo, hi in ((0, 512), (512, F)):
                    nc.tensor.matmul(a_ps[:, lo:hi], ones1[:], af16[:, lo:hi],
                                     start=True, stop=True)
                dtile = ptile("dtile", [P, F], dt.float16)
                nc.vector.tensor_sub(dtile[:], i0zf[:], a_ps[:])
                zstate.update(ai16=ai16, dtile=dtile)

            def bview(t):
                return t.unsqueeze(1).broadcast_to([P, F, K])

            def cview(t):
                return t[:].unsqueeze(2).broadcast_to([P, F, K])

            def z_piece_2():
                wz, dtile = zstate["wz"], zstate["dtile"]
                iota = cst[:, 2:2 + K]              # 0,1,2,3
                omw = ptile("omw", [P, F], dt.float16)
                nc.vector.tensor_scalar(omw[:], wz[:], -1.0, 1.0,
                                        Alu.mult, Alu.add)
                e0 = ptile("e0", [P, F, K], dt.float16)
                nc.vector.tensor_tensor(e0[:], bview(iota), cview(dtile),
                                        Alu.is_equal)
                m0 = ptile("m0", [P, F, K], dt.float16)
                nc.vector.tensor_mul(m0[:], e0[:], cview(omw))
                zstate.update(m0=m0)

            def z_piece_3():
                wz, dtile, m0 = zstate["wz"], zstate["dtile"], zstate["m0"]
                iom1 = cst[:, 6:6 + K]              # -1,0,1,2
                wz16 = ptile("wz16", [P, F], dt.float16)
                nc.vector.tensor_copy(wz16[:], wz[:])
                e1 = ptile("e1", [P, F, K], dt.float16)
                nc.vector.tensor_tensor(e1[:], bview(iom1), cview(dtile),
                                        Alu.is_equal)
                m1 = ptile("m1", [P, F, K], dt.float16)
                nc.vector.tensor_mul(m1[:], e1[:], cview(wz16))
                nc.vector.tensor_add(Wpt[:], m0[:], m1[:])

            def z_piece_4():
                ai16 = zstate["ai16"]
                # slab idx: j = K*t + r -> band j%16, col j//16
                # band b, col k: idx = a[4k + b//4] + b%4
                ph = ptile("ph", [1, 4, F // 4], dt.int16)
                ai4 = ai16[:].rearrange("o (k q) -> o k q", q=4)
                for p4 in range(4):
                    nc.vector.tensor_copy(ph[:, p4, :], ai4[:, :, p4])
                stg16 = ptile("stg16", [16, SLABCOLS], dt.int16)
                for b in range(16):
                    nc.sync.dma_start(stg16[b:b + 1, :], ph[:, b // 4, :])
                stg16a = ptile("stg16a", [16, SLABCOLS], dt.int16)
                nc.vector.tensor_add(stg16a[:], stg16[:],
                                     rc[:].broadcast_to([16, SLABCOLS]))
                for b in range(8):
                    nc.sync.dma_start(slabidx[16 * b:16 * (b + 1), :],
                                      stg16a[:])

            def z_piece_5():
                nc.gpsimd.dma_gather(
                    slabT[:], tbz, slabidx[:], NSLAB, NSLAB, ELEM,
                    elem_step=ELEM, queue_num=0, single_packet=False,
                    transpose=True)

            # ---------- main loop ----------
            with (
                tc.tile_pool(name="stg", bufs=3) as stg_pool,
                tc.tile_pool(name="gath", bufs=3) as gath_pool,
                tc.tile_pool(name="mid", bufs=2) as mid_pool,
                tc.tile_pool(name="mpool", bufs=8) as m_pool,
                tc.tile_pool(name="aps", bufs=1, space="PSUM") as aps_pool,
                tc.tile_pool(name="uvp", bufs=2, space="PSUM") as uv_pool,
                tc.tile_pool(name="utp", bufs=2, space="PSUM") as ut_pool,
            ):
                with tc.tile_critical():
                    nc.gpsimd.load_library(lib_mlp)

                zpieces = [z_piece_0, lambda: z_piece_1(aps_pool), z_piece_2,
                           z_piece_3, z_piece_4, z_piece_5]

                def z_tail(c, m):
                    # uv[r, t, :] = sum_c slab[c, r] * m[c, :], per tile
                    ut = ut_pool.tile([P, CHUNK_T, K], dt.float16, tag="ut")
                    for s8 in range(2):
                        uv = uv_pool.tile([K, CHUNK_T // 2, P], dt.float32,
                                          tag="uv")
                        for tt in range(CHUNK_T // 2):
                            t = s8 * (CHUNK_T // 2) + tt
                            T = CHUNK_T * c + t
                            nc.tensor.matmul(
                                uv[:, tt, :],
                                slabT[0:C, 0, K * T:K * (T + 1)],
                                m[0:C, P * t:P * (t + 1)],
                                start=True, stop=True)
                        uvs = mid_pool.tile([K, CHUNK_T // 2, P], dt.float16,
                                            tag="uvs")
                        nc.scalar.copy(uvs[:], uv[:])
                        for tt in range(CHUNK_T // 2):
                            t = s8 * (CHUNK_T // 2) + tt
                            nc.tensor.transpose(ut[:, t, :], uvs[:, tt, :],
                                                ident[0:K, 0:K])

                    # out[p, f] = sum_r W[p, f, r] * ut[p, f, r]
                    cs = slice(CHUNK_T * c, CHUNK_T * (c + 1))
                    pr = mid_pool.tile([P, CHUNK_T, K], dt.float16, tag="pr")
                    nc.vector.tensor_mul(pr[:], Wpt[:, cs, :], ut[:])
                    h2 = mid_pool.tile([P, CHUNK_T, 2], dt.float16, tag="h2")
                    nc.vector.tensor_add(h2[:], pr[:, :, 0:2], pr[:, :, 2:4])
                    nc.vector.tensor_tensor(
                        out_full[:, cs].unsqueeze(2),
                        h2[:, :, 0:1], h2[:, :, 1:2], Alu.add)

                stg_tiles = {}
                # group 0 staged in two column slices (early first gather)
                stg0 = stg_pool.tile([P, 2, PHI], dt.int16, tag="stg")
                for b in range(8):
                    nc.sync.dma_start(stg0[16 * b:16 * (b + 1), :, 0:SL],
                                      idx_all[0:16, :, 0:SL])
                for b in range(8):
                    nc.sync.dma_start(stg0[16 * b:16 * (b + 1), :, SL:PHI],
                                      idx_all[0:16, :, SL:PHI])
                stg_tiles[0] = stg0
                backlog = []
                for c in range(N_CHUNKS):
                    grp = c // CHUNKS_PER_GROUP
                    sub = c % CHUNKS_PER_GROUP

                    if sub == 0 and grp > 0:
                        stg = stg_pool.tile([P, 2, PHI], dt.int16, tag="stg")
                        src = idx_all[16 * grp:16 * (grp + 1), :, :]
                        for b in range(8):
                            nc.sync.dma_start(
                                stg[16 * b:16 * (b + 1), :, :], src)
                        stg_tiles[grp] = stg
                    stg = stg_tiles[grp]

                    # y/x gathers in c-layout (transpose mode)
                    az = []
                    for a, tbl in ((0, tby), (1, tbx)):
                        g = gath_pool.tile([P, 2, CHUNK_PTS], dt.float16,
                                           tag=f"g{a}")
                        idxs = stg[:, a, SUBCOLS * sub:SUBCOLS * (sub + 1)]
                        nc.gpsimd.dma_gather(
                            g[:], tbl, idxs, CHUNK_PTS, CHUNK_PTS, ELEM,
                            elem_step=ELEM, queue_num=0, single_packet=False,
                            transpose=True)
                        az.append(g)

                    # m = gy * gx  (c-layout, comps on partitions 0..95)
                    m = m_pool.tile([P, CHUNK_PTS], dt.float16, tag="m")
                    nc.vector.tensor_mul(m[:], az[0][:, 0, :], az[1][:, 0, :])

                    if c < len(zpieces):
                        zpieces[c]()
                        backlog.append((c, m))
                        if c == len(zpieces) - 1:
                            for bc, bm in backlog:
                                z_tail(bc, bm)
                            backlog = None
                    else:
                        z_tail(c, m)

                nc.sync.dma_start(out_d, out_full[:])

    nc.compile()
    return nc


def _build_tables(line_z, line_y, line_x):
    qs = np.arange(NQ)
    i0 = np.minimum(qs // S, R - 2)
    w = (qs / S - i0).astype(np.float32)[None, :]
    ups = []
    for L in (line_y, line_x):
        Lf = np.asarray(L, dtype=np.float32)
        Uq = Lf[:, i0] * (1.0 - w) + Lf[:, i0 + 1] * w
        row = np.zeros((NQ, ELEM), dtype=np.float16)
        row[:, 0:C] = Uq.T.astype(np.float16)
        ups.append(row)
    Lz = np.asarray(line_z, dtype=np.float32)
    tz = np.zeros((RPAD, ELEM), dtype=np.float16)
    tz[:R, 0:C] = Lz.T.astype(np.float16)
    return ups[0], ups[1], tz


def _host_prep(in_tensor, line_z, line_y, line_x):
    pts = np.ascontiguousarray(in_tensor.reshape(-1, 3).astype(np.float32))
    tby, tbx, tbz = _build_tables(line_z, line_y, line_x)

    consts = np.zeros((P, 10), np.float16)
    consts[:, 0] = 1.0
    consts[:, 2:6] = np.arange(K, dtype=np.float16)[None, :]
    consts[:, 6:10] = (np.arange(K, dtype=np.float16) - 1.0)[None, :]
    rcon = (np.arange(16, dtype=np.int16) % K)[:, None]

    in_maps, orders = [], []
    for k in range(N_CORES):
        shard = pts[k * N_CORE:(k + 1) * N_CORE]
        posz = (shard[:, 2] + 1.0) * 0.5 * (R - 1)
        i0 = np.clip(np.floor(posz), 0, R - 1).astype(np.int64)
        order = np.argsort(i0, kind="stable")
        srt = shard[order]
        izs = i0[order].reshape(F, P)
        span = izs.max(1) - izs.min(1)
        assert span.max() <= K - 2, f"z tile span {span.max()} needs K>{K}"
        orders.append(order)

        # y/x coords wrapped-16: point j of group g at [16g + j%16, j//16]
        yx = srt[:, [1, 0]]
        pw16 = np.ascontiguousarray(
            yx.reshape(GROUPS, PHI, 16, 2).transpose(0, 2, 1, 3)
            .reshape(P, PHI * 2))
        # z coords in pt-layout n = 128*f + p
        pwz = np.ascontiguousarray(srt[:, 2].reshape(F, P).T)
        in_maps.append({
            "pw16": pw16, "pwz": pwz,
            "tby": tby, "tbx": tbx, "tbz": tbz,
            "consts": consts, "rcon": rcon,
            "onesrow": np.ones((1, P), np.float16),
        })
    return in_maps, orders


def _unshard(results, orders):
    outs = []
    for k in range(N_CORES):
        w = np.asarray(results[k]["out"])            # (128, 768), sorted order
        srt = w.T.reshape(-1)
        restored = np.empty_like(srt)
        restored[orders[k]] = srt
        outs.append(restored)
    return np.concatenate(outs).reshape(4096, 192).astype(np.float32)


def kernel(in_tensor, line_z, line_y, line_x):
    global _BUILT
    from concourse.bass_utils import run_bass_kernel_spmd

    if _BUILT is None:
        _BUILT = _build_nc()
    nc = _BUILT
    in_maps, orders = _host_prep(np.asarray(in_tensor), np.asarray(line_z),
                                 np.asarray(line_y), np.asarray(line_x))
    res = run_bass_kernel_spmd(nc, in_maps, list(range(N_CORES)))
    return _unshard(res.results, orders)


# revision 17
# speedup vs baseline: 1.0127x; 1.0127x over previous
"""CPDecoding (embedding_lookup) Trainium2 kernel.

out[n] = sum_c fz[c,n]*fy[c,n]*fx[c,n], where f* is a 1-D linear
interpolation (grid_sample, align_corners=True) of a (96, 512) line table
at per-point coordinates.

Per core (98304 points, z-sorted on host, flat order n = 2048*chunk + j):
  - y/x axes: 64x pre-upsampled fp16 tables (linear interp pre-evaluated,
    so no interp FMA on device); dma_gather(transpose=True) lands rows in
    c-layout [component-partition, point-free]. m = gy*gx on DVE (one
    fp16 2x multiply -- no other elementwise work).
  - z axis + the 96-component reduction fused on the PE: points are
    sorted by iz = floor(pos_z), so each 128-point tile spans <= 2
    consecutive z rows. Per tile: uv[r, :] = sum_c Lz[c, a_t+r] * m[c, :]
    (K=96 matmul, M=4), with the 4-row slab fetched once per tile by a
    small slab dma_gather. A tiny PE transpose puts uv in pt-layout and
    DVE dots it with 4-wide interp weights
    W[p,f,r] = (r==d)*(1-w) + (r==d+1)*w.  No z gather, no reduce tree.
  - The z-side setup (index chains, weight build, slab fetch) is emitted
    in slices across the first chunk bodies so the y/x gather stream
    starts within a few microseconds.
  - Host un-permutes the output.
"""

import numpy as np

N_CORES = 8
N_TOTAL = 4096 * 192
N_CORE = N_TOTAL // N_CORES      # 98304 points per core
P = 128
F = N_CORE // P                  # 768 tiles of 128 points
GROUPS = 8
PHI = N_CORE // 16 // GROUPS     # 768 idx-cols per group (wrapped-16)
C = 96
R = 512
S = 64                           # y/x upsample factor
NQ = (R - 1) * S + 1             # 32705
ELEM = 256                       # fp16 elems per row (512B, transpose mode)
K = 4                            # z slab rows per tile
RPAD = R + 8                     # z table rows incl. OOB pad
CHUNK_T = 16                     # tiles per chunk
CHUNK_PTS = P * CHUNK_T          # 2048 points
N_CHUNKS = F // CHUNK_T          # 48
CHUNKS_PER_GROUP = N_CHUNKS // GROUPS  # 6
SUBCOLS = CHUNK_PTS // 16        # 128 idx cols per chunk
QSCALE = (R - 1) * S
NSLAB = K * F                    # 3072 slab-row fetches
SLABCOLS = NSLAB // 16           # 192

_BUILT = None


def _build_nc():
    import concourse.bacc as bacc
    import concourse.tile as tile
    from concourse import mybir
    from concourse.library_config import mlp as lib_mlp
    from concourse.masks import make_identity

    dt = mybir.dt
    Alu = mybir.AluOpType

    nc = bacc.Bacc("TRN2", target_bir_lowering=False, debug=False,
                   num_devices=N_CORES, num_swdge_queues=1)

    pw16 = nc.dram_tensor("pw16", [P, PHI * 2], dt.float32,
                          kind="ExternalInput").ap()         # y,x wrapped-16
    pwz = nc.dram_tensor("pwz", [P, F], dt.float32,
                         kind="ExternalInput").ap()          # z pt-layout
    tby = nc.dram_tensor("tby", [NQ, ELEM], dt.float16,
                         kind="ExternalInput").ap()
    tbx = nc.dram_tensor("tbx", [NQ, ELEM], dt.float16,
                         kind="ExternalInput").ap()
    tbz = nc.dram_tensor("tbz", [RPAD, ELEM], dt.float16,
                         kind="ExternalInput").ap()          # exact z rows
    consts = nc.dram_tensor("consts", [P, 10], dt.float16,
                            kind="ExternalInput").ap()
    rcon = nc.dram_tensor("rcon", [16, 1], dt.int16,
                          kind="ExternalInput").ap()
    onesrow = nc.dram_tensor("onesrow", [1, P], dt.float16,
                             kind="ExternalInput").ap()
    out_d = nc.dram_tensor("out", [P, F], dt.float32,
                           kind="ExternalOutput").ap()

    with tile.TileContext(nc) as tc:
        with tc.tile_pool(name="persist", bufs=1) as pp:
            idx_all = pp.tile([P, 2, PHI], dt.int16, tag="idx")
            out_full = pp.tile([P, F], dt.float32, tag="out")
            Wpt = pp.tile([P, F, K], dt.float16, tag="wpt")
            slabT = pp.tile([P, 2, NSLAB], dt.float16, tag="slabT")
            slabidx = pp.tile([P, SLABCOLS], dt.int16, tag="slabidx")
            cst = pp.tile([P, 10], dt.float16, tag="cst")
            nc.sync.dma_start(cst[:], consts)
            rc = pp.tile([16, 1], dt.int16, tag="rcon")
            nc.sync.dma_start(rc[:], rcon)
            ones1 = pp.tile([1, P], dt.float16, tag="ones1")
            nc.sync.dma_start(ones1[:], onesrow)
            ident = pp.tile([P, P], dt.float16, tag="ident")
            make_identity(nc, ident[:])

            def ptile(nm, shape, dtype=dt.float32):
                return pp.tile(shape, dtype, tag=nm, name=nm)

            def floor_chain(pool_tmp, src, shape, scale, bias, hi):
                """clamp(floor(src*scale + bias), 0, hi) as fp32."""
                pos = pool_tmp(f"pos{shape[-1]}_{hi}", shape)
                nc.vector.tensor_scalar(pos[:], src, scale, bias,
                                        Alu.mult, Alu.add)
                ii = pool_tmp(f"ii{hi}", shape, dt.int32)
                nc.vector.tensor_copy(ii[:], pos[:])
                i0f = pool_tmp(f"i0f{hi}", shape)
                nc.vector.tensor_copy(i0f[:], ii[:])
                neg = pool_tmp(f"neg{hi}", shape)
                nc.vector.tensor_tensor(neg[:], pos[:], i0f[:], Alu.is_lt)
                i0a = pool_tmp(f"i0a{hi}", shape)
                nc.vector.tensor_sub(i0a[:], i0f[:], neg[:])
                i0c = pool_tmp(f"i0c{hi}", shape)
                nc.vector.tensor_scalar(i0c[:], i0a[:], float(hi), 0.0,
                                        Alu.min, Alu.max)
                return pos, i0c

            # ---------- upfront: y/x upsampled indices (wrapped-16) -------
            with tc.tile_pool(name="setup", bufs=1) as sp:
                def stmp(nm, shape, dtype=dt.float32):
                    return sp.tile(shape, dtype, tag=nm, name=nm)

                s16 = stmp("c16", [P, PHI * 2])
                nc.sync.dma_start(s16[:], pw16)
                _, q64 = floor_chain(stmp, s16[:], [P, PHI * 2],
                                     QSCALE / 2.0, QSCALE / 2.0 + 0.5,
                                     NQ - 1)
                q_3d = q64[:].rearrange("p (f k) -> p f k", k=2)
                for a in range(2):
                    nc.vector.tensor_copy(idx_all[:, a, :], q_3d[:, :, a])

            # ---------- z-side setup, emitted in slices ----------
            zstate = {}

            def z_piece_0():
                sz = ptile("cz", [P, F])
                nc.sync.dma_start(sz[:], pwz)
                t1z = ptile("t1z", [P, F])
                nc.vector.tensor_scalar(t1z[:], sz[:], 1.0, 0.5,
                                        Alu.add, Alu.mult)
                posz, i0zf = floor_chain(ptile, t1z[:], [P, F],
                                         float(R - 1), 0.0, R - 2)
                wz = ptile("wz", [P, F])
                nc.vector.tensor_sub(wz[:], posz[:], i0zf[:])
                zstate.update(i0zf=i0zf, wz=wz)

            def z_piece_1(aps_pool):
                i0zf = zstate["i0zf"]
                af16 = ptile("af16", [1, F], dt.float16)
                nc.vector.tensor_copy(af16[:], i0zf[0:1, :])
                ai16 = ptile("ai16", [1, F], dt.int16)
                nc.vector.tensor_copy(ai16[:], i0zf[0:1, :])
                a_ps = aps_pool.tile([P, F], dt.float32, tag="aps")
                for lo, hi in ((0, 512), (512, F)):
                    nc.tensor.matmul(a_ps[:, lo:hi], ones1[:], af16[:, lo:hi],
                                     start=True, stop=True)
                dtile = ptile("dtile", [P, F], dt.float16)
                nc.vector.tensor_sub(dtile[:], i0zf[:], a_ps[:])
                zstate.update(ai16=ai16, dtile=dtile)

            def bview(t):
                return t.unsqueeze(1).broadcast_to([P, F, K])

            def cview(t):
                return t[:].unsqueeze(2).broadcast_to([P, F, K])

            def z_piece_2():
                wz, dtile = zstate["wz"], zstate["dtile"]
                iota = cst[:, 2:2 + K]              # 0,1,2,3
                omw = ptile("omw", [P, F], dt.float16)
                nc.vector.tensor_scalar(omw[:], wz[:], -1.0, 1.0,
                                        Alu.mult, Alu.add)
                e0 = ptile("e0", [P, F, K], dt.float16)
                nc.vector.tensor_tensor(e0[:], bview(iota), cview(dtile),
                                        Alu.is_equal)
                m0 = ptile("m0", [P, F, K], dt.float16)
                nc.vector.tensor_mul(m0[:], e0[:], cview(omw))
                zstate.update(m0=m0)

            def z_piece_3():
                wz, dtile, m0 = zstate["wz"], zstate["dtile"], zstate["m0"]
                iom1 = cst[:, 6:6 + K]              # -1,0,1,2
                wz16 = ptile("wz16", [P, F], dt.float16)
                nc.vector.tensor_copy(wz16[:], wz[:])
                e1 = ptile("e1", [P, F, K], dt.float16)
                nc.vector.tensor_tensor(e1[:], bview(iom1), cview(dtile),
                                        Alu.is_equal)
                m1 = ptile("m1", [P, F, K], dt.float16)
                nc.vector.tensor_mul(m1[:], e1[:], cview(wz16))
                nc.vector.tensor_add(Wpt[:], m0[:], m1[:])

            def z_piece_4():
                ai16 = zstate["ai16"]
                # slab idx: j = K*t + r -> band j%16, col j//16
                # band b, col k: idx = a[4k + b//4] + b%4
                ph = ptile("ph", [1, 4, F // 4], dt.int16)
                ai4 = ai16[:].rearrange("o (k q) -> o k q", q=4)
                for p4 in range(4):
                    nc.vector.tensor_copy(ph[:, p4, :], ai4[:, :, p4])
                stg16 = ptile("stg16", [16, SLABCOLS], dt.int16)
                for b in range(16):
                    nc.sync.dma_start(stg16[b:b + 1, :], ph[:, b // 4, :])
                stg16a = ptile("stg16a", [16, SLABCOLS], dt.int16)
                nc.vector.tensor_add(stg16a[:], stg16[:],
                                     rc[:].broadcast_to([16, SLABCOLS]))
                for b in range(8):
                    nc.sync.dma_start(slabidx[16 * b:16 * (b + 1), :],
                                      stg16a[:])

            def z_piece_5():
                nc.gpsimd.dma_gather(
                    slabT[:], tbz, slabidx[:], NSLAB, NSLAB, ELEM,
                    elem_step=ELEM, queue_num=0, single_packet=False,
                    transpose=True)

            # ---------- main loop ----------
            with (
                tc.tile_pool(name="stg", bufs=3) as stg_pool,
                tc.tile_pool(name="gath", bufs=3) as gath_pool,
                tc.tile_pool(name="mid", bufs=2) as mid_pool,
                tc.tile_pool(name="mpool", bufs=8) as m_pool,
                tc.tile_pool(name="aps", bufs=1, space="PSUM") as aps_pool,
                tc.tile_pool(name="uvp", bufs=2, space="PSUM") as uv_pool,
                tc.tile_pool(name="utp", bufs=2, space="PSUM") as ut_pool,
            ):
                with tc.tile_critical():
                    nc.gpsimd.load_library(lib_mlp)

                zpieces = [z_piece_0, lambda: z_piece_1(aps_pool), z_piece_2,
                           z_piece_3, z_piece_4, z_piece_5]

                def z_tail(c, m):
                    # uv[r, t, :] = sum_c slab[c, r] * m[c, :], per tile
                    ut = ut_pool.tile([P, CHUNK_T, K], dt.float16, tag="ut")
                    for s8 in range(2):
                        uv = uv_pool.tile([K, CHUNK_T // 2, P], dt.float32,
                                          tag="uv")
                        for tt in range(CHUNK_T // 2):
                            t = s8 * (CHUNK_T // 2) + tt
                            T = CHUNK_T * c + t
                            nc.tensor.matmul(
                                uv[:, tt, :],
                                slabT[0:C, 0, K * T:K * (T + 1)],
                                m[0:C, P * t:P * (t + 1)],
                                start=True, stop=True)
                        uvs = mid_pool.tile([K, CHUNK_T // 2, P], dt.float16,
                                            tag="uvs")
                        nc.scalar.copy(uvs[:], uv[:])
                        for tt in range(CHUNK_T // 2):
                            t = s8 * (CHUNK_T // 2) + tt
                            nc.tensor.transpose(ut[:, t, :], uvs[:, tt, :],
                                                ident[0:K, 0:K])

                    # out[p, f] = sum_r W[p, f, r] * ut[p, f, r]
                    cs = slice(CHUNK_T * c, CHUNK_T * (c + 1))
                    pr = mid_pool.tile([P, CHUNK_T, K], dt.float16, tag="pr")
                    nc.vector.tensor_mul(pr[:], Wpt[:, cs, :], ut[:])
                    h2 = mid_pool.tile([P, CHUNK_T, 2], dt.float16, tag="h2")
                    nc.vector.tensor_add(h2[:], pr[:, :, 0:2], pr[:, :, 2:4])
                    nc.vector.tensor_tensor(
                        out_full[:, cs].unsqueeze(2),
                        h2[:, :, 0:1], h2[:, :, 1:2], Alu.add)

                stg_tiles = {}
                backlog = []
                for c in range(N_CHUNKS):
                    grp = c // CHUNKS_PER_GROUP
                    sub = c % CHUNKS_PER_GROUP

                    if sub == 0:
                        stg = stg_pool.tile([P, 2, PHI], dt.int16, tag="stg")
                        src = idx_all[16 * grp:16 * (grp + 1), :, :]
                        for b in range(8):
                            nc.sync.dma_start(
                                stg[16 * b:16 * (b + 1), :, :], src)
                        stg_tiles[grp] = stg
                    stg = stg_tiles[grp]

                    # y/x gathers in c-layout (transpose mode)
                    az = []
                    for a, tbl in ((0, tby), (1, tbx)):
                        g = gath_pool.tile([P, 2, CHUNK_PTS], dt.float16,
                                           tag=f"g{a}")
                        idxs = stg[:, a, SUBCOLS * sub:SUBCOLS * (sub + 1)]
                        nc.gpsimd.dma_gather(
                            g[:], tbl, idxs, CHUNK_PTS, CHUNK_PTS, ELEM,
                            elem_step=ELEM, queue_num=0, single_packet=False,
                            transpose=True)
                        az.append(g)

                    # m = gy * gx  (c-layout, comps on partitions 0..95)
                    m = m_pool.tile([P, CHUNK_PTS], dt.float16, tag="m")
                    nc.vector.tensor_mul(m[:], az[0][:, 0, :], az[1][:, 0, :])

                    if c < len(zpieces):
                        zpieces[c]()
                        backlog.append((c, m))
                        if c == len(zpieces) - 1:
                            for bc, bm in backlog:
                                z_tail(bc, bm)
                            backlog = None
                    else:
                        z_tail(c, m)

                nc.sync.dma_start(out_d, out_full[:])

    nc.compile()
    return nc


def _build_tables(line_z, line_y, line_x):
    qs = np.arange(NQ)
    i0 = np.minimum(qs // S, R - 2)
    w = (qs / S - i0).astype(np.float32)[None, :]
    ups = []
    for L in (line_y, line_x):
        Lf = np.asarray(L, dtype=np.float32)
        Uq = Lf[:, i0] * (1.0 - w) + Lf[:, i0 + 1] * w
        row = np.zeros((NQ, ELEM), dtype=np.float16)
        row[:, 0:C] = Uq.T.astype(np.float16)
        ups.append(row)
    Lz = np.asarray(line_z, dtype=np.float32)
    tz = np.zeros((RPAD, ELEM), dtype=np.float16)
    tz[:R, 0:C] = Lz.T.astype(np.float16)
    return ups[0], ups[1], tz


def _host_prep(in_tensor, line_z, line_y, line_x):
    pts = np.ascontiguousarray(in_tensor.reshape(-1, 3).astype(np.float32))
    tby, tbx, tbz = _build_tables(line_z, line_y, line_x)

    consts = np.zeros((P, 10), np.float16)
    consts[:, 0] = 1.0
    consts[:, 2:6] = np.arange(K, dtype=np.float16)[None, :]
    consts[:, 6:10] = (np.arange(K, dtype=np.float16) - 1.0)[None, :]
    rcon = (np.arange(16, dtype=np.int16) % K)[:, None]

    in_maps, orders = [], []
    for k in range(N_CORES):
        shard = pts[k * N_CORE:(k + 1) * N_CORE]
        posz = (shard[:, 2] + 1.0) * 0.5 * (R - 1)
        i0 = np.clip(np.floor(posz), 0, R - 1).astype(np.int64)
        order = np.argsort(i0, kind="stable")
        srt = shard[order]
        izs = i0[order].reshape(F, P)
        span = izs.max(1) - izs.min(1)
        assert span.max() <= K - 2, f"z tile span {span.max()} needs K>{K}"
        orders.append(order)

        # y/x coords wrapped-16: point j of group g at [16g + j%16, j//16]
        yx = srt[:, [1, 0]]
        pw16 = np.ascontiguousarray(
            yx.reshape(GROUPS, PHI, 16, 2).transpose(0, 2, 1, 3)
            .reshape(P, PHI * 2))
        # z coords in pt-layout n = 128*f + p
        pwz = np.ascontiguousarray(srt[:, 2].reshape(F, P).T)
        in_maps.append({
            "pw16": pw16, "pwz": pwz,
            "tby": tby, "tbx": tbx, "tbz": tbz,
            "consts": consts, "rcon": rcon,
            "onesrow": np.ones((1, P), np.float16),
        })
    return in_maps, orders


def _unshard(results, orders):
    outs = []
    for k in range(N_CORES):
        w = np.asarray(results[k]["out"])            # (128, 768), sorted order
        srt = w.T.reshape(-1)
        restored = np.empty_like(srt)
        restored[orders[k]] = srt
        outs.append(restored)
    return np.concatenate(outs).reshape(4096, 192).astype(np.float32)


def kernel(in_tensor, line_z, line_y, line_x):
    global _BUILT
    from concourse.bass_utils import run_bass_kernel_spmd

    if _BUILT is None:
        _BUILT = _build_nc()
    nc = _BUILT
    in_maps, orders = _host_prep(np.asarray(in_tensor), np.asarray(line_z),
                                 np.asarray(line_y), np.asarray(line_x))
    res = run_bass_kernel_spmd(nc, in_maps, list(range(N_CORES)))
    return _unshard(res.results, orders)
